# revision 1
# baseline (speedup 1.0000x reference)
"""FM bi-interaction (embedding_lookup) Trainium2 kernel.

out[n, k] = 0.5 * ((x @ E)^2 - (x*x) @ (E*E))[n, k] * mask[n]
mask[n] = 1 if n in train_idx else 0

Sharding: data-parallel over the 20000 input rows, 2500 rows per core on 8
NeuronCores; the [10000, 32] embedding table is replicated. Per core, x is
uploaded in f-major (transposed) layout so the contraction dim lands on SBUF
partitions with fully contiguous 1 MB DMAs. The train_idx mask is built
on-device without dynamic DMA (disabled in this runtime): each core receives
the indices that fall in its row range (rebased, padded with sentinel ROWS);
per 128-index batch, DVE computes eq[p, n] = (iota[n] == idx[p]) and an
all-ones [128, 32] matmul reduces eq over partitions into match counts,
replicated across the 32 output partitions; min(count, 1) * 0.5 gives the
half-mask applied in the epilogue.

Matmuls run in float32r (full-rate fp32 on the PE, ~1.5e-4 relative
accuracy), accumulating 80 f-tiles of 125 contraction rows into PSUM per
500-column output chunk.
"""

import math
import sys

if "/opt/trn_rl_repo" not in sys.path:
    sys.path.insert(0, "/opt/trn_rl_repo")

import numpy as np

N_ROWS = 20000
F = 10000
EK = 32
CORES = 8
ROWS = N_ROWS // CORES  # 2500 rows per core
NCHUNK = 500
CHUNKS = ROWS // NCHUNK  # 5
FP = 125  # contraction rows per f-tile (125 * 80 = 10000)
FTILES = F // FP  # 80

_PROGRAM_CACHE: dict = {}


def _build_program(k_idx: int):
    """Per-core Bass program. k_idx = number of 128-index scatter batches."""
    import concourse.bass as bass
    import concourse.mybir as mybir
    import concourse.tile as tile
    from concourse import bacc

    f32 = mybir.dt.float32
    f32r = mybir.dt.float32r
    bf16 = mybir.dt.bfloat16

    nc = bacc.Bacc("TRN2", target_bir_lowering=False, debug=False)
    xt = nc.dram_tensor("xt", [F, ROWS], f32r, kind="ExternalInput")
    emb = nc.dram_tensor("emb", [F, EK], f32r, kind="ExternalInput")
    # train indices as floats, padded with ROWS (matches nothing in iota)
    idxf = nc.dram_tensor("idxf", [128, k_idx], f32, kind="ExternalInput")
    iota_in = nc.dram_tensor("iota", [128, ROWS], f32, kind="ExternalInput")
    outT = nc.dram_tensor("outT", [EK, ROWS], f32, kind="ExternalOutput")

    with tile.TileContext(nc) as tc:
        with (
            tc.tile_pool(name="wpool", bufs=1) as wpool,
            tc.tile_pool(name="mpool", bufs=1) as mpool,
            tc.tile_pool(name="xpool", bufs=10) as xpool,
            tc.tile_pool(name="qpool", bufs=4) as qpool,
            tc.tile_pool(name="opool", bufs=2) as opool,
        ):
            # Embedding table (and its elementwise square) as stationary
            # operands: 80 f-tiles of [125, 32] each.
            e_sb = wpool.tile([FP, FTILES, EK], f32r)
            nc.sync.dma_start(
                out=e_sb[:], in_=emb[:].rearrange("(a p) k -> p a k", p=FP)
            )
            e2_sb = wpool.tile([FP, FTILES, EK], f32r)
            nc.vector.tensor_mul(e2_sb[:], e_sb[:], e_sb[:])

            # Half-mask (values 0 / 0.5), built without dynamic DMA:
            # eq[p, n] = (n == idx[p, j]) in bf16 (0/1 exact); an all-ones
            # bf16 [128, 32] matmul sums eq over partitions (match count) and
            # replicates the row across 32 output partitions. Counts for
            # chunks 0-3 are packed into ONE persistent PSUM bank at
            # partition offsets 32c (col-tiling); chunk 4 uses a second bank.
            # min(count, 1) * 0.5 is fused into the epilogue read.
            iota_sb = mpool.tile([128, ROWS], f32)
            nc.sync.dma_start(out=iota_sb[:], in_=iota_in[:])
            idx_sb = mpool.tile([128, k_idx], f32)
            nc.sync.dma_start(out=idx_sb[:], in_=idxf[:])
            ones_sb = mpool.tile([128, EK], bf16)
            nc.gpsimd.memset(ones_sb[:], 1.0)

            QUAD = 4
            MSLOTS = FTILES // QUAD  # mask batches that fit in chunk 0
            ps_ctx = tc.tile_pool(name="pspool", bufs=3, space="PSUM")
            pspool = ps_ctx.__enter__()
            eq_ctx = tc.tile_pool(name="eqpool", bufs=2)
            eqpool = eq_ctx.__enter__()
            psMaskA = pspool.tile([128, 512], f32, space="PSUM", bufs=1)
            psMaskB = pspool.tile([EK, 512], f32, space="PSUM", bufs=1)

            def emit_mask_batch(j):
                eq = eqpool.tile([128, ROWS], bf16, name="eq")
                nc.vector.tensor_tensor(
                    out=eq[:],
                    in0=iota_sb[:],
                    in1=idx_sb[:, j : j + 1].broadcast_to([128, ROWS]),
                    op=mybir.AluOpType.is_equal,
                )
                for cc in range(CHUNKS):
                    tgt = (
                        psMaskA[32 * cc : 32 * cc + 32, :NCHUNK]
                        if cc < 4
                        else psMaskB[:, :NCHUNK]
                    )
                    nc.tensor.matmul(
                        tgt,
                        ones_sb[:],
                        eq[:, cc * NCHUNK : (cc + 1) * NCHUNK],
                        start=(j == 0),
                        stop=(j == k_idx - 1),
                        tile_position=(0, 32 * cc if cc < 4 else 0),
                    )

            interleaved = k_idx <= MSLOTS
            if not interleaved:
                for j in range(k_idx):
                    emit_mask_batch(j)

            for c in range(CHUNKS):
                ns = slice(c * NCHUNK, (c + 1) * NCHUNK)
                psL = pspool.tile([EK, NCHUNK], f32, space="PSUM")
                psR = pspool.tile([EK, NCHUNK], f32, space="PSUM")
                for q in range(FTILES // QUAD):
                    t0 = q * QUAD
                    xt_sb = xpool.tile([FP, QUAD, NCHUNK], f32r)
                    nc.sync.dma_start(
                        out=xt_sb[:],
                        in_=xt[t0 * FP : (t0 + QUAD) * FP, ns].rearrange(
                            "(a p) n -> p a n", p=FP
                        ),
                    )
                    xq_sb = qpool.tile([FP, QUAD, NCHUNK], f32r)
                    nc.vector.tensor_mul(xq_sb[:], xt_sb[:], xt_sb[:])
                    for h in range(QUAD):
                        t = t0 + h
                        nc.tensor.matmul(
                            psL[:],
                            e_sb[:, t, :],
                            xt_sb[:, h, :],
                            start=(t == 0),
                            stop=(t == FTILES - 1),
                        )
                        nc.tensor.matmul(
                            psR[:],
                            e2_sb[:, t, :],
                            xq_sb[:, h, :],
                            start=(t == 0),
                            stop=(t == FTILES - 1),
                        )
                    if interleaved and c == 0 and q < k_idx:
                        emit_mask_batch(q)
                # out = (L*L - R) * (min(count, 1) * 0.5)
                msrc = (
                    psMaskA[32 * c : 32 * c + 32, :NCHUNK]
                    if c < 4
                    else psMaskB[:, :NCHUNK]
                )
                msb = opool.tile([EK, NCHUNK], f32)
                nc.vector.tensor_scalar(
                    out=msb[:],
                    in0=msrc,
                    scalar1=1.0,
                    scalar2=0.5,
                    op0=mybir.AluOpType.min,
                    op1=mybir.AluOpType.mult,
                )
                lsb = opool.tile([EK, NCHUNK], f32)
                nc.vector.tensor_copy(lsb[:], psL[:])
                osb = opool.tile([EK, NCHUNK], f32)
                nc.vector.tensor_mul(osb[:], lsb[:], lsb[:])
                nc.vector.tensor_sub(osb[:], osb[:], psR[:])
                nc.vector.tensor_mul(osb[:], osb[:], msb[:])
                nc.sync.dma_start(out=outT[:, ns], in_=osb[:])
            eq_ctx.__exit__(None, None, None)
            ps_ctx.__exit__(None, None, None)

    nc.compile()
    return nc


def _get_program(k_idx: int):
    if k_idx not in _PROGRAM_CACHE:
        _PROGRAM_CACHE[k_idx] = _build_program(k_idx)
    return _PROGRAM_CACHE[k_idx]


def _prepare_in_maps(input, emb_weight, train_idx):
    x = np.asarray(input, dtype=np.float32)
    e = np.ascontiguousarray(np.asarray(emb_weight, dtype=np.float32))
    idx = np.asarray(train_idx).astype(np.int64)

    per_core_idx = []
    max_cnt = 1
    for c in range(CORES):
        lo = c * ROWS
        sel = idx[(idx >= lo) & (idx < lo + ROWS)] - lo
        sel = sel.astype(np.int32)
        per_core_idx.append(sel)
        max_cnt = max(max_cnt, len(sel))
    k_idx = max(1, math.ceil(max_cnt / 128))
    pad_len = 128 * k_idx

    iota = np.ascontiguousarray(
        np.broadcast_to(np.arange(ROWS, dtype=np.float32), (128, ROWS))
    )
    in_maps = []
    for c in range(CORES):
        sel = per_core_idx[c]
        padded = np.full(pad_len, ROWS, dtype=np.float32)  # ROWS matches nothing
        padded[: len(sel)] = sel.astype(np.float32)
        xt = np.ascontiguousarray(x[c * ROWS : (c + 1) * ROWS, :].T)
        in_maps.append(
            {
                "xt": xt,
                "emb": e,
                "idxf": padded.reshape(128, k_idx),
                "iota": iota,
            }
        )
    return in_maps, k_idx


def run_sharded(input, emb_weight, train_idx, trace: bool = False):
    """Run on 8 cores; returns (full_output, BassKernelResults)."""
    from concourse.bass_utils import run_bass_kernel_spmd

    in_maps, k_idx = _prepare_in_maps(input, emb_weight, train_idx)
    nc = _get_program(k_idx)
    res = run_bass_kernel_spmd(
        nc, in_maps, core_ids=list(range(CORES)), trace=trace
    )
    out = np.empty((N_ROWS, EK), dtype=np.float32)
    for c in range(CORES):
        out[c * ROWS : (c + 1) * ROWS, :] = res.results[c]["outT"].T
    return out, res


def kernel(input, emb_weight, train_idx):
    out, _ = run_sharded(input, emb_weight, train_idx)
    return out



# revision 2
# speedup vs baseline: 4.4535x; 4.4535x over previous
"""FM bi-interaction (embedding_lookup) Trainium2 kernel — v2.

out[n, k] = 0.5 * ((x @ E)^2 - (x*x) @ (E*E))[n, k] * mask[n]
mask[n] = 1 if n in train_idx else 0

Strategy (all sharding/prep is host-side, inside kernel()):
- Only rows that appear in train_idx produce nonzero output (~55% of rows
  for the target distribution: 16000 draws with replacement from 20000).
  Dedup train_idx, gather just those rows, and scatter results back into a
  zero output. The on-device mask disappears entirely.
- The 0.5 factor and the mask are folded into the embedding table: with
  E' = sqrt(0.5) * E, (x@E')^2 - (x*x)@(E'*E') = 0.5*((x@E)^2 - (x*x)@(E*E)).
- x is uploaded in bf16 (halves HBM traffic; rel-err ~3e-3 << 2e-2 budget)
  in f-major [F, R_pad] layout per core so every x DMA is a 1.5 MB transfer
  with 3 KB contiguous lines.
- PE: the K=32 output only fills 32 of 128 PE columns, so four matmuls run
  concurrently via col-tiling (tile_position=(0, 32j)): groups 0/1 hold
  L/R partial sums for even f-tile pairs, groups 2/3 for odd pairs. The
  epilogue adds the two partial L's (and R's), then out = L^2 - R.
- x^2 is computed on device, split between VectorE and ScalarE.

Rows per core are padded to a multiple of 512 (one PSUM bank of output
columns per 512-row chunk); the Bass program is cached per chunk count.
"""

import math
import sys

if "/opt/trn_rl_repo" not in sys.path:
    sys.path.insert(0, "/opt/trn_rl_repo")

import numpy as np
import ml_dtypes

BF16 = ml_dtypes.bfloat16

N_ROWS = 20000
F = 10000
EK = 32
CORES = 8
FP = 125  # contraction rows per f-tile
FTILES = F // FP  # 80
QUAD = 4
NQ = FTILES // QUAD  # 20
NCHUNK = 512  # output columns per PSUM bank

_PROGRAM_CACHE: dict = {}


def _build_program(nch: int):
    """Per-core Bass program for nch chunks of 512 gathered rows."""
    import concourse.mybir as mybir
    import concourse.tile as tile
    from concourse import bacc

    f32 = mybir.dt.float32
    bf16 = mybir.dt.bfloat16
    rpad = nch * NCHUNK

    nc = bacc.Bacc("TRN2", target_bir_lowering=False, debug=False)
    xt = nc.dram_tensor("xt", [F, rpad], bf16, kind="ExternalInput")
    embP = nc.dram_tensor("embP", [FP, FTILES * EK], bf16, kind="ExternalInput")
    outT = nc.dram_tensor("outT", [EK, rpad], f32, kind="ExternalOutput")

    with tile.TileContext(nc) as tc:
        with (
            tc.tile_pool(name="wpool", bufs=1) as wpool,
            tc.tile_pool(name="xpool", bufs=3) as xpool,
            tc.tile_pool(name="qpool", bufs=3) as qpool,
            tc.tile_pool(name="opool", bufs=2) as opool,
            tc.tile_pool(name="pspool", bufs=1, space="PSUM") as pspool,
        ):
            # Embedding table, pre-scaled by sqrt(0.5) and pre-rearranged to
            # [FP, FTILES*EK] on host: one fully contiguous DMA.
            e_sb = wpool.tile([FP, FTILES * EK], bf16)
            nc.sync.dma_start(out=e_sb[:], in_=embP[:])
            e2_sb = wpool.tile([FP, FTILES * EK], bf16)
            nc.vector.tensor_mul(e2_sb[:], e_sb[:], e_sb[:])

            def wslice(sb, t):
                return sb[:, t * EK : (t + 1) * EK]

            # One PSUM bank per chunk; partition groups hold the four
            # col-tiled accumulators: [0:32]=L(t%4 in 0,1), [32:64]=R(same),
            # [64:96]=L(t%4 in 2,3), [96:128]=R(same).
            ps = [
                pspool.tile([128, NCHUNK], f32, space="PSUM", name=f"ps{c}", bufs=1)
                for c in range(nch)
            ]

            for q in range(NQ):
                t0 = QUAD * q
                xt_sb = xpool.tile([FP, QUAD, rpad], bf16, name="xt_sb")
                nc.sync.dma_start(
                    out=xt_sb[:],
                    in_=xt[t0 * FP : (t0 + QUAD) * FP, :].rearrange(
                        "(a p) n -> p a n", p=FP
                    ),
                )
                xq_sb = qpool.tile([FP, QUAD, rpad], bf16, name="xq_sb")
                # split the squaring across VectorE and ScalarE
                nc.vector.tensor_mul(
                    xq_sb[:, 0:2, :], xt_sb[:, 0:2, :], xt_sb[:, 0:2, :]
                )
                nc.scalar.square(xq_sb[:, 2:4, :], xt_sb[:, 2:4, :])

                for c in range(nch):
                    ns = slice(c * NCHUNK, (c + 1) * NCHUNK)
                    for h in range(2):
                        t = t0 + h  # even pair -> groups 0 (L) and 1 (R)
                        u = t0 + 2 + h  # odd pair -> groups 2 (L) and 3 (R)
                        nc.tensor.matmul(
                            ps[c][0:32, :],
                            wslice(e_sb, t),
                            xt_sb[:, h, ns],
                            start=(t == 0),
                            stop=(t == FTILES - 3),
                            tile_position=(0, 0),
                        )
                        nc.tensor.matmul(
                            ps[c][32:64, :],
                            wslice(e2_sb, t),
                            xq_sb[:, h, ns],
                            start=(t == 0),
                            stop=(t == FTILES - 3),
                            tile_position=(0, 32),
                        )
                        nc.tensor.matmul(
                            ps[c][64:96, :],
                            wslice(e_sb, u),
                            xt_sb[:, 2 + h, ns],
                            start=(u == 2),
                            stop=(u == FTILES - 1),
                            tile_position=(0, 64),
                        )
                        nc.tensor.matmul(
                            ps[c][96:128, :],
                            wslice(e2_sb, u),
                            xq_sb[:, 2 + h, ns],
                            start=(u == 2),
                            stop=(u == FTILES - 1),
                            tile_position=(0, 96),
                        )

            # Epilogue: L = g0 + g2, R = g1 + g3, out = L*L - R.
            for c in range(nch):
                ns = slice(c * NCHUNK, (c + 1) * NCHUNK)
                lsb = opool.tile([EK, NCHUNK], f32, name="lsb")
                nc.scalar.activation(
                    lsb[:], ps[c][0:32, :], mybir.ActivationFunctionType.Copy
                )
                nc.vector.tensor_add(lsb[:], lsb[:], ps[c][64:96, :])
                rsb = opool.tile([EK, NCHUNK], f32, name="rsb")
                nc.scalar.activation(
                    rsb[:], ps[c][32:64, :], mybir.ActivationFunctionType.Copy
                )
                nc.vector.tensor_add(rsb[:], rsb[:], ps[c][96:128, :])
                osb = opool.tile([EK, NCHUNK], f32, name="osb")
                nc.scalar.square(osb[:], lsb[:])
                nc.vector.tensor_sub(osb[:], osb[:], rsb[:])
                nc.sync.dma_start(out=outT[:, ns], in_=osb[:])

    nc.compile()
    return nc


def _get_program(nch: int):
    if nch not in _PROGRAM_CACHE:
        _PROGRAM_CACHE[nch] = _build_program(nch)
    return _PROGRAM_CACHE[nch]


def _prepare_in_maps(input, emb_weight, train_idx):
    x = np.asarray(input, dtype=np.float32)
    e = np.asarray(emb_weight, dtype=np.float32)
    idx = np.asarray(train_idx).astype(np.int64)

    uniq = np.unique(idx)
    u = len(uniq)
    per_core = max(1, math.ceil(u / CORES))
    nch = max(1, math.ceil(per_core / NCHUNK))
    rpad = nch * NCHUNK

    # embedding: scale by sqrt(0.5) (folds the 0.5 and keeps L^2-R exact),
    # rearrange to [FP, FTILES*EK] so the device DMA is contiguous.
    es = (e * math.sqrt(0.5)).reshape(FTILES, FP, EK).transpose(1, 0, 2)
    embP = np.ascontiguousarray(es.reshape(FP, FTILES * EK).astype(BF16))

    groups = []
    in_maps = []
    for c in range(CORES):
        sel = uniq[c * per_core : (c + 1) * per_core]
        groups.append(sel)
        xt = np.zeros((F, rpad), dtype=BF16)
        if len(sel):
            xt[:, : len(sel)] = x[sel].astype(BF16).T
        in_maps.append({"xt": xt, "embP": embP})
    return in_maps, nch, groups


def run_sharded(input, emb_weight, train_idx, trace: bool = False):
    """Run on 8 cores; returns (full_output, BassKernelResults)."""
    from concourse.bass_utils import run_bass_kernel_spmd

    in_maps, nch, groups = _prepare_in_maps(input, emb_weight, train_idx)
    nc = _get_program(nch)
    res = run_bass_kernel_spmd(nc, in_maps, core_ids=list(range(CORES)), trace=trace)
    out = np.zeros((N_ROWS, EK), dtype=np.float32)
    for c in range(CORES):
        sel = groups[c]
        if len(sel):
            out[sel, :] = res.results[c]["outT"].T[: len(sel)]
    return out, res


def kernel(input, emb_weight, train_idx):
    out, _ = run_sharded(input, emb_weight, train_idx)
    return out


# revision 9
# speedup vs baseline: 7.9911x; 1.7944x over previous
"""FM bi-interaction (embedding_lookup) Trainium2 kernel — v2.

out[n, k] = 0.5 * ((x @ E)^2 - (x*x) @ (E*E))[n, k] * mask[n]
mask[n] = 1 if n in train_idx else 0

Strategy (all sharding/prep is host-side, inside kernel()):
- Only rows that appear in train_idx produce nonzero output (~55% of rows
  for the target distribution: 16000 draws with replacement from 20000).
  Dedup train_idx, gather just those rows, and scatter results back into a
  zero output. The on-device mask disappears entirely.
- The 0.5 factor and the mask are folded into the embedding table: with
  E' = sqrt(0.5) * E, (x@E')^2 - (x*x)@(E'*E') = 0.5*((x@E)^2 - (x*x)@(E*E)).
- x is uploaded in bf16 (halves HBM traffic; rel-err ~3e-3 << 2e-2 budget)
  in f-major [F, R_pad] layout per core so every x DMA is a 1.5 MB transfer
  with 3 KB contiguous lines.
- PE: the K=32 output only fills 32 of 128 PE columns, so four matmuls run
  concurrently via col-tiling (tile_position=(0, 32j)): groups 0/1 hold
  L/R partial sums for even f-tile pairs, groups 2/3 for odd pairs. The
  epilogue adds the two partial L's (and R's), then out = L^2 - R.
- x^2 is computed on device, split between VectorE and ScalarE.

Rows per core are padded to a multiple of 512 (one PSUM bank of output
columns per 512-row chunk); the Bass program is cached per chunk count.
"""

import math
import sys

if "/opt/trn_rl_repo" not in sys.path:
    sys.path.insert(0, "/opt/trn_rl_repo")

import numpy as np
import ml_dtypes

BF16 = ml_dtypes.bfloat16

N_ROWS = 20000
F = 10000
EK = 32
CORES = 8
FP = 125  # contraction rows per f-tile
FTILES = F // FP  # 80
QUAD = 4
NQ = FTILES // QUAD  # 20
NCHUNK = 512  # output columns per PSUM bank
CPAD = 128  # per-core row count is padded to a multiple of this

_PROGRAM_CACHE: dict = {}


def _build_program(rpad: int):
    """Per-core Bass program for rpad gathered rows (multiple of CPAD)."""
    import concourse.mybir as mybir
    import concourse.tile as tile
    from concourse import bacc

    f32 = mybir.dt.float32
    bf16 = mybir.dt.bfloat16
    # output chunks: full 512-col PSUM banks plus one partial bank
    chunk_cols = [NCHUNK] * (rpad // NCHUNK)
    if rpad % NCHUNK:
        chunk_cols.append(rpad % NCHUNK)
    nch = len(chunk_cols)

    nc = bacc.Bacc("TRN2", target_bir_lowering=False, debug=False)
    xt = nc.dram_tensor("xt", [F, rpad], bf16, kind="ExternalInput")
    embP = nc.dram_tensor("embP", [FP, FTILES * EK], bf16, kind="ExternalInput")
    outT = nc.dram_tensor("outT", [EK, rpad], f32, kind="ExternalOutput")

    with tile.TileContext(nc) as tc:
        with (
            tc.tile_pool(name="wpool", bufs=1) as wpool,
            tc.tile_pool(name="xpool", bufs=4) as xpool,
            tc.tile_pool(name="qpool", bufs=4) as qpool,
            tc.tile_pool(name="opool", bufs=2) as opool,
            tc.tile_pool(name="pspool", bufs=1, space="PSUM") as pspool,
        ):
            # Embedding table, pre-scaled by sqrt(0.5) and pre-rearranged to
            # [FP, FTILES*EK] on host: one fully contiguous DMA.
            e_sb = wpool.tile([FP, FTILES * EK], bf16)
            nc.sync.dma_start(out=e_sb[:], in_=embP[:])
            e2_sb = wpool.tile([FP, FTILES * EK], bf16)
            nc.vector.tensor_mul(e2_sb[:], e_sb[:], e_sb[:])

            def wslice(sb, t):
                return sb[:, t * EK : (t + 1) * EK]

            # One PSUM bank per chunk; partition groups hold the four
            # col-tiled accumulators: [0:32]=L(t%4 in 0,1), [32:64]=R(same),
            # [64:96]=L(t%4 in 2,3), [96:128]=R(same).
            ps = [
                pspool.tile([128, NCHUNK], f32, space="PSUM", name=f"ps{c}", bufs=1)
                for c in range(nch)
            ]

            for q in range(NQ):
                t0 = QUAD * q
                xt_sb = xpool.tile([FP, QUAD, rpad], bf16, name="xt_sb")
                nc.sync.dma_start(
                    out=xt_sb[:],
                    in_=xt[t0 * FP : (t0 + QUAD) * FP, :].rearrange(
                        "(a p) n -> p a n", p=FP
                    ),
                )
                xq_sb = qpool.tile([FP, QUAD, rpad], bf16, name="xq_sb")
                # split the squaring across VectorE and ScalarE
                nc.vector.tensor_mul(
                    xq_sb[:, 0:2, :], xt_sb[:, 0:2, :], xt_sb[:, 0:2, :]
                )
                nc.scalar.square(xq_sb[:, 2:4, :], xt_sb[:, 2:4, :])

                for c in range(nch):
                    ns = slice(c * NCHUNK, c * NCHUNK + chunk_cols[c])
                    cs = slice(0, chunk_cols[c])
                    for h in range(2):
                        t = t0 + h  # even pair -> groups 0 (L) and 1 (R)
                        u = t0 + 2 + h  # odd pair -> groups 2 (L) and 3 (R)
                        nc.tensor.matmul(
                            ps[c][0:32, cs],
                            wslice(e_sb, t),
                            xt_sb[:, h, ns],
                            start=(t == 0),
                            stop=(t == FTILES - 3),
                            tile_position=(0, 0),
                            skip_group_check=True,
                        )
                        nc.tensor.matmul(
                            ps[c][32:64, cs],
                            wslice(e2_sb, t),
                            xq_sb[:, h, ns],
                            start=(t == 0),
                            stop=(t == FTILES - 3),
                            tile_position=(0, 32),
                            skip_group_check=True,
                        )
                        nc.tensor.matmul(
                            ps[c][64:96, cs],
                            wslice(e_sb, u),
                            xt_sb[:, 2 + h, ns],
                            start=(u == 2),
                            stop=(u == FTILES - 1),
                            tile_position=(0, 64),
                            skip_group_check=True,
                        )
                        nc.tensor.matmul(
                            ps[c][96:128, cs],
                            wslice(e2_sb, u),
                            xq_sb[:, 2 + h, ns],
                            start=(u == 2),
                            stop=(u == FTILES - 1),
                            tile_position=(0, 96),
                            skip_group_check=True,
                        )

            # Epilogue: L = g0 + g2, R = g1 + g3, out = L*L - R.
            for c in range(nch):
                ns = slice(c * NCHUNK, c * NCHUNK + chunk_cols[c])
                cs = slice(0, chunk_cols[c])
                lsb = opool.tile([EK, NCHUNK], f32, name="lsb")
                nc.scalar.activation(
                    lsb[:, cs], ps[c][0:32, cs], mybir.ActivationFunctionType.Copy
                )
                nc.vector.tensor_add(lsb[:, cs], lsb[:, cs], ps[c][64:96, cs])
                rsb = opool.tile([EK, NCHUNK], f32, name="rsb")
                nc.scalar.activation(
                    rsb[:, cs], ps[c][32:64, cs], mybir.ActivationFunctionType.Copy
                )
                nc.vector.tensor_add(rsb[:, cs], rsb[:, cs], ps[c][96:128, cs])
                osb = opool.tile([EK, NCHUNK], f32, name="osb")
                nc.scalar.square(osb[:, cs], lsb[:, cs])
                nc.vector.tensor_sub(osb[:, cs], osb[:, cs], rsb[:, cs])
                nc.sync.dma_start(out=outT[:, ns], in_=osb[:, cs])

    nc.compile()
    return nc


def _get_program(rpad: int):
    if rpad not in _PROGRAM_CACHE:
        _PROGRAM_CACHE[rpad] = _build_program(rpad)
    return _PROGRAM_CACHE[rpad]


def _prepare_in_maps(input, emb_weight, train_idx):
    x = np.asarray(input, dtype=np.float32)
    e = np.asarray(emb_weight, dtype=np.float32)
    idx = np.asarray(train_idx).astype(np.int64)

    uniq = np.unique(idx)
    u = len(uniq)
    per_core = max(1, math.ceil(u / CORES))
    rpad = CPAD * math.ceil(per_core / CPAD)

    # embedding: scale by sqrt(0.5) (folds the 0.5 and keeps L^2-R exact),
    # rearrange to [FP, FTILES*EK] so the device DMA is contiguous.
    es = (e * math.sqrt(0.5)).reshape(FTILES, FP, EK).transpose(1, 0, 2)
    embP = np.ascontiguousarray(es.reshape(FP, FTILES * EK).astype(BF16))

    groups = []
    in_maps = []
    for c in range(CORES):
        sel = uniq[c * per_core : (c + 1) * per_core]
        groups.append(sel)
        xt = np.zeros((F, rpad), dtype=BF16)
        if len(sel):
            xt[:, : len(sel)] = x[sel].astype(BF16).T
        in_maps.append({"xt": xt, "embP": embP})
    return in_maps, rpad, groups


def run_sharded(input, emb_weight, train_idx, trace: bool = False):
    """Run on 8 cores; returns (full_output, BassKernelResults)."""
    from concourse.bass_utils import run_bass_kernel_spmd

    in_maps, rpad, groups = _prepare_in_maps(input, emb_weight, train_idx)
    nc = _get_program(rpad)
    res = run_bass_kernel_spmd(nc, in_maps, core_ids=list(range(CORES)), trace=trace)
    out = np.zeros((N_ROWS, EK), dtype=np.float32)
    for c in range(CORES):
        sel = groups[c]
        if len(sel):
            out[sel, :] = res.results[c]["outT"].T[: len(sel)]
    return out, res


def kernel(input, emb_weight, train_idx):
    out, _ = run_sharded(input, emb_weight, train_idx)
    return out
